# revision 25
# baseline (speedup 1.0000x reference)
"""Trainium2 Bass kernel for nn_BiologicalMemory (retrieval_knn).

Full-input contract: kernel(**inputs) takes the complete unsharded inputs and
returns the complete [4096] output. Internally shards across 8 NeuronCores:
  - memory_bank row-sharded fp16 (1024 rows per core) for the score pass
  - full memory_bank replicated fp16 per core for candidate-row gather
  - W_dec.T column-sharded fp16 (each core produces 512 output elements),
    host-pre-permuted into the exact SBUF layout so the DMA is contiguous
  - query replicated (pre-broadcast to 128 partitions, fp16)

Score pass: dots on DVE (affine_mul_reduce), row sum-of-squares on ACT
(Square+accumulate), streaming the fp16 shard. The scalar (ACT) engine
carries NO bulk DMA dispatches -- the Tile scheduler statically interleaves
an engine's dma_starts with its compute, and a backed-up ring blocks the
Squares for ~20us -- so memory tiles are half-split across the sync (HWDGE)
and gpsimd (SWDGE) rings instead.

Ranking uses the monotone transform u = dots*|dots| * (imp*exp(-.001*age))^2
/ ssq, which needs no sqrt (no ACT table switch) and drops the ||q|| common
factor. Local top-8 (value, global row id) pairs are AllGathered as a
64-byte payload; every core reduces the 64 candidates by score threshold,
gathers the candidate rows from its local full-bank copy via indirect DMA,
mean-pools with a thresholded-weight matmul, and decodes its 512-wide
output slice with fp16 matmuls straight from SBUF. Dummy PE matmuls spanning
the collective window keep the tensor engine's HAM clock un-throttled for
the decode.
"""

import numpy as np

import concourse.bass as bass
import concourse.mybir as mybir
import concourse.tile as tile
from concourse import bacc
from concourse.bass import ts
from concourse.bass_utils import run_bass_kernel_spmd
from concourse.masks import make_identity

DIM = 4096
CAP = 8192
NCORES = 8
RPC = CAP // NCORES   # rows per core        (1024)
OPC = DIM // NCORES   # output elems / core  (512)
K = 8                 # top_k
NT = RPC // 128       # row tiles per core   (8)
DC = DIM // 128       # d-chunks             (32)
NC64 = NCORES * K     # candidate count      (64)
CCB = 2 * K           # collective payload, f32 elems (vals + idx)
NWARM = 64            # PE warmer matmuls spanning the collective window

F32 = mybir.dt.float32
F16 = mybir.dt.float16
U32 = mybir.dt.uint32
AF = mybir.ActivationFunctionType
ALU = mybir.AluOpType


def _build_nc():
    nc = bacc.Bacc(None, num_devices=NCORES, debug=False)
    _emit(nc)
    nc.compile()
    return nc


def _emit(nc):
    mfull = nc.dram_tensor("mfull", [CAP, DIM], F16, kind="ExternalInput")
    qb_d = nc.dram_tensor("qb", [128, (NT + 1) * DIM], F16, kind="ExternalInput")
    impa = nc.dram_tensor("impa", [128, NT], F32, kind="ExternalInput")
    agev = nc.dram_tensor("agev", [128, NT], F32, kind="ExternalInput")
    wt = nc.dram_tensor("wt", [128, DC * OPC], F16, kind="ExternalInput")
    bcv = nc.dram_tensor("bcv", [1, OPC], F32, kind="ExternalInput")
    coff_d = nc.dram_tensor("coff", [1, 1], U32, kind="ExternalInput")
    out = nc.dram_tensor("out", [1, OPC], F32, kind="ExternalOutput")

    with tile.TileContext(nc) as tc:
        with (
            tc.tile_pool(name="persist", bufs=1) as pp,
            tc.tile_pool(name="mtp", bufs=8) as mtp,
            tc.tile_pool(name="scr", bufs=1) as scrp,
            tc.tile_pool(name="small", bufs=1) as sp,
            tc.tile_pool(name="psum", bufs=1, space="PSUM") as psp,
            tc.tile_pool(name="dram", bufs=1, space="DRAM") as dp,
        ):
            # Each ring DMA pays ~1.8us of serialized completion overhead, so
            # the query + memory stream is host-staged in the exact SBUF
            # layout (slot 0 = query broadcast, slots 1..8 = row tiles) and
            # shipped as 4 contiguous half-DMAs per ring: tile-arrival
            # cadence then matches the DVE dot rate with minimal overhead.
            HD = DIM // 2
            dots8 = sp.tile([128, NT], F32, name="dots8")
            ss8 = sp.tile([128, NT], F32, name="ss8")
            mball = pp.tile([128, NT + 1, DIM], F16, name="mball")
            imp_sb = sp.tile([128, NT], F32, name="imp_sb")
            age_sb = sp.tile([128, NT], F32, name="age_sb")
            ms = qb_d.rearrange("p (s d) -> p s d", d=DIM)
            for g, (lo, hi) in enumerate([(0, 3), (3, 5), (5, 7), (7, 9)]):
                nc.sync.dma_start(mball[:, lo:hi, :HD], ms[:, lo:hi, :HD])
                nc.gpsimd.dma_start(mball[:, lo:hi, HD:], ms[:, lo:hi, HD:])
                if g == 1:
                    nc.sync.dma_start(imp_sb, impa[:, :])
                    nc.sync.dma_start(age_sb, agev[:, :])
            qb = mball[:, 0, :]
            m_tiles = [mball[:, 1 + t, :] for t in range(NT)]

            # decoder slice, host-pre-permuted: wt[p, c*OPC+o], so each half
            # is one fully-contiguous 2MB transfer.
            wt_sb = pp.tile([128, DC, OPC], F16, name="wt_sb")
            wt_flat = wt_sb.rearrange("p c o -> p (c o)")
            nc.sync.dma_start(wt_flat[:, : DC * OPC // 2], wt[:, : DC * OPC // 2])
            nc.gpsimd.dma_start(wt_flat[:, DC * OPC // 2 :], wt[:, DC * OPC // 2 :])

            ident = pp.tile([128, 128], F32, name="ident")
            make_identity(nc, ident)
            coff = sp.tile([1, 1], U32, name="coff")
            nc.gpsimd.dma_start(coff, coff_d[:, :])
            bc_sb = sp.tile([1, OPC], F32, name="bc_sb")
            nc.gpsimd.dma_start(bc_sb, bcv[:, :])

            for t in range(NT):
                m_t = m_tiles[t]
                # NOTE: tensor_tensor_reduce passes CoreSim but hangs TRN2
                # hardware here -- keep the custom affine_mul_reduce.
                dscr = scrp.tile([128, DIM], F16, name="dscr", tag="dvescr")
                nc.vector.affine_mul_reduce(
                    out=dscr,
                    accum_out=dots8[:, t : t + 1],
                    in0=m_t,
                    in1=qb,
                    scale=1.0,
                    bias=0.0,
                )
                ascr = scrp.tile([128, DIM], F16, name="ascr", tag="actscr")
                nc.scalar.activation(
                    ascr, m_t, AF.Square, accum_out=ss8[:, t : t + 1]
                )

            # ie2 = (importance * exp(-0.001*age))^2
            ie8 = sp.tile([128, NT], F32, name="ie8")
            nc.scalar.activation(ie8, age_sb, AF.Exp, scale=-0.001)
            nc.vector.tensor_mul(ie8, ie8, imp_sb)
            ie2 = sp.tile([128, NT], F32, name="ie2")
            nc.vector.tensor_mul(ie2, ie8, ie8)

            # ---- ranking scores u = dots*|dots| * ie^2 / ssq  [128, 8]
            rss = sp.tile([128, NT], F32, name="rss")
            nc.vector.reciprocal(rss, ss8)
            ad8 = sp.tile([128, NT], F32, name="ad8")
            nc.scalar.activation(ad8, dots8, AF.Abs)
            u8 = sp.tile([128, NT], F32, name="u8")
            nc.vector.tensor_mul(u8, dots8, ad8)
            nc.vector.tensor_mul(u8, u8, ie2)
            nc.vector.tensor_mul(u8, u8, rss)

            # ---- flatten scores to [1, 1024] in row order (r = t*128 + p)
            st_ps = psp.tile([NT, 128], F32, name="st_ps", tag="pT")
            nc.tensor.transpose(st_ps, u8, ident)
            st = sp.tile([NT, 128], F32, name="st")
            nc.vector.tensor_copy(st, st_ps)
            sflat = sp.tile([1, RPC], F32, name="sflat")
            nc.sync.dma_start(sflat, st)

            # ---- local top-8 (values + global row ids), packed in one tile
            # so the collective payload ships with a single DMA
            packed = sp.tile([1, 2 * K], F32, name="packed")
            nc.vector.max(out=packed[:, :K], in_=sflat)
            idx8 = sp.tile([1, 8], U32, name="idx8")
            nc.vector.max_index(out=idx8, in_max=packed[:, :K], in_values=sflat)
            # global row id = local id | c*1024 (exact: local id < 1024)
            nc.vector.tensor_scalar(
                packed[:, K:].bitcast(U32), idx8, coff[:, :1], None,
                op0=ALU.bitwise_or,
            )

            # ---- AllGather the 64-byte candidate payload
            cc_in = dp.tile([CCB], F32, name="cc_in")
            cc_out = dp.tile([NCORES * CCB], F32, name="cc_out", addr_space="Shared")
            nc.sync.dma_start(cc_in[:].unsqueeze(0), packed)
            nc.gpsimd.collective_compute(
                "AllGather",
                ALU.bypass,
                replica_groups=[list(range(NCORES))],
                ins=[cc_in.opt()],
                outs=[cc_out.opt()],
            )
            cc8 = cc_out.rearrange("(c x) -> c x", x=CCB)

            # ---- global top-8 among the 64 candidates, by score threshold.
            # The 8th max is broadcast to 64 partitions with a degenerate PE
            # matmul (ones[1,64]^T @ thr[1,1]): gpsimd compute here would pay
            # a multi-us Q7 drain right before the gather.
            vals64 = sp.tile([1, NC64], F32, name="vals64")
            nc.sync.dma_start(vals64, cc8[:, :K])
            vals64c = sp.tile([NC64, 1], F32, name="vals64c")
            nc.sync.dma_start(vals64c, cc8[:, :K])
            idx64c = sp.tile([NC64, 1], U32, name="idx64c")
            nc.scalar.dma_start(idx64c, cc8[:, K:].bitcast(U32))
            gv8 = sp.tile([1, 8], F32, name="gv8")
            nc.vector.max(out=gv8, in_=vals64)
            ones64 = sp.tile([1, NC64], F32, name="ones64")
            nc.vector.memset(ones64, 1.0)
            thr_ps = psp.tile([NC64, 1], F32, name="thr_ps", tag="pB")
            nc.tensor.matmul(
                thr_ps, lhsT=ones64, rhs=gv8[0:1, 7:8], start=True, stop=True
            )
            w64 = sp.tile([NC64, 1], F16, name="w64")
            nc.vector.tensor_scalar(
                w64, vals64c, thr_ps[:, :1], 1.0 / K, op0=ALU.is_ge, op1=ALU.mult
            )

            # ---- gather the 64 candidate rows from the local full bank
            rows64 = pp.tile([NC64, DIM], F16, name="rows64")
            nc.gpsimd.indirect_dma_start(
                out=rows64[:],
                out_offset=None,
                in_=mfull[:, :],
                in_offset=bass.IndirectOffsetOnAxis(ap=idx64c[:, :1], axis=0),
            )

            # ---- retrieved = w64 . rows64, in [128, 32] layout
            ret_ps = psp.tile([128, DC], F32, name="ret_ps", tag="pA")
            for c in range(DC):
                nc.tensor.matmul(
                    ret_ps[:, c : c + 1],
                    lhsT=rows64[:, ts(c, 128)],
                    rhs=w64,
                    start=True,
                    stop=True,
                )
            ret16 = sp.tile([128, DC], F16, name="ret16")
            nc.vector.tensor_copy(ret16, ret_ps)

            # ---- decode: out_slice = retrieved @ W_dec[slice].T + b[slice]
            out_ps = psp.tile([1, OPC], F32, name="out_ps", tag="pout")
            for c in range(DC):
                nc.tensor.matmul(
                    out_ps,
                    lhsT=ret16[:, c : c + 1],
                    rhs=wt_sb[:, c, :],
                    start=(c == 0),
                    stop=(c == DC - 1),
                )
            out_sb = sp.tile([1, OPC], F32, name="out_sb")
            nc.vector.tensor_add(out_sb, out_ps, bc_sb)
            nc.sync.dma_start(out[:, :], out_sb)


_NC_CACHE = {}


def _get_nc():
    if "nc" not in _NC_CACHE:
        _NC_CACHE["nc"] = _build_nc()
    return _NC_CACHE["nc"]


def _make_in_maps(query, memory_bank, importance, age, W_dec, b_dec):
    query = np.asarray(query, dtype=np.float32)
    memory_bank = np.asarray(memory_bank, dtype=np.float32)
    importance = np.asarray(importance, dtype=np.float32)
    age = np.asarray(age, dtype=np.float32)
    W_dec = np.asarray(W_dec, dtype=np.float32)
    b_dec = np.asarray(b_dec, dtype=np.float32)

    mf16 = np.ascontiguousarray(memory_bank.astype(np.float16))
    q16 = query.astype(np.float16)
    in_maps = []
    for c in range(NCORES):
        rs = slice(c * RPC, (c + 1) * RPC)
        os = slice(c * OPC, (c + 1) * OPC)
        # mstage[p, 0, :] = q; mstage[p, 1+t, :] = bank row t*128+p of shard
        mstage = np.empty((128, NT + 1, DIM), dtype=np.float16)
        mstage[:, 0, :] = q16[None, :]
        mstage[:, 1:, :] = mf16[rs].reshape(NT, 128, DIM).transpose(1, 0, 2)
        # wt[p, c*OPC + o] = W_dec[os.start + o, c*128 + p]
        wtp = np.ascontiguousarray(
            W_dec[os, :].T.astype(np.float16).reshape(DC, 128, OPC)
            .transpose(1, 0, 2).reshape(128, DC * OPC)
        )
        in_maps.append(
            {
                "mfull": mf16,
                "qb": np.ascontiguousarray(mstage.reshape(128, (NT + 1) * DIM)),
                "impa": np.ascontiguousarray(importance[rs].reshape(NT, 128).T),
                "agev": np.ascontiguousarray(age[rs].reshape(NT, 128).T),
                "wt": wtp,
                "bcv": np.ascontiguousarray(b_dec[os].reshape(1, OPC)),
                "coff": np.array([[c * RPC]], dtype=np.uint32),
            }
        )
    return in_maps


def run(inputs, trace=False, **run_kwargs):
    """Build (cached), run on 8 cores, gather. Returns (output, BassKernelResults)."""
    assert int(inputs.get("top_k", K)) == K
    nc = _get_nc()
    in_maps = _make_in_maps(
        inputs["query"],
        inputs["memory_bank"],
        inputs["importance"],
        inputs["age"],
        inputs["W_dec"],
        inputs["b_dec"],
    )
    res = run_bass_kernel_spmd(
        nc, in_maps, core_ids=list(range(NCORES)), trace=trace, **run_kwargs
    )
    out = np.concatenate(
        [res.results[c]["out"].reshape(OPC) for c in range(NCORES)]
    ).astype(np.float32)
    return out, res


def kernel(**inputs) -> np.ndarray:
    out, _ = run(inputs, trace=False)
    return out
